# revision 21
# baseline (speedup 1.0000x reference)
"""Trainium2 Bass kernel for nn_Block_56538949484919 (dense transformer block).

Sharding: data-parallel over batch B=4 x 2-way split of the query rows
(sequence dim) => 8 cores, no collectives. Each core receives its batch's
h_shared / h_private pre-transposed to feature-major layout [C, L] with the
sequence axis rolled so that its query half is always columns [0, L/2).
K/V are computed over the full (rolled) sequence on every core; attention is
permutation-invariant over keys, so the roll does not change the result.

Host-side precomputation (cheap, O(C^2)):
  - Wvv = Wv @ Wvt (value transform folded into one matrix)
  - LayerNorm affine (w, b) folded into the following weight matrix/bias
  - all fp8 weights pre-scaled by 64 before e4m3 quantization (the raw
    0.02-scale weights sit mostly in the e4m3 subnormal range; scaling
    recovers the full 3-bit mantissa). The 1/64 rescale folds into the
    existing PSUM-evacuation scale slots for free.
  - bcp folded into the streamed h_private residual (hpRb).

Numerics (gate relmax < 2e-2; isolation-swept on a host fp8 simulator):
  - All projection/MLP matmuls run fp8e4m3 DoubleRow (0.5 cyc/row).
  - Wcp is single-term (Whi @ y_hi): its quantization noise rides on the
    softmax-averaged y and is negligible. Wfc / Wproj stay 3-term hi/lo
    (Whi*xhi + Wlo*xhi + Whi*xlo): each dropped correction there costs
    ~1e-2 relmax, too close to the gate.
  - softmax exp is split across engines: ACT runs exact Exp for a share of
    score tiles; DVE computes the rest with a Schraudolph-style integer
    approximation (round(logit*8/ln2 + 56 + sigma) as int8, bitcast e4m3),
    whose per-key relative error washes out in the softmax normalization.
  - Residual stream h_private and all PSUM accumulation stay fp32.

Schedule (the span was dependency-bound, not engine-bound):
  - K/Q for all 8 head pairs are projected up front; the attention loop is
    then outer over the two 512-query i-chunks. After chunk ic finishes,
    its whole tail (Wcp -> h1 -> ln2 -> MLP -> output DMA, mostly PE work)
    runs while the attention of chunk ic+1 (mostly ACT/DVE exp work)
    proceeds concurrently.
  - The attention jp loop is software-pipelined: the score matmuls of
    round jp+1 are emitted before the yps-accumulate of round jp, so the
    in-order PE queue does not stall on the exp latency.
  - PSUM budget (8 banks): scores 2x2 + yps 2 + shared tail pool 2.
  - softmax 1/den: DVE reciprocal -> DRAM bounce -> broadcast DMA across
    the 64 head partitions (no PE/ACT time, no PSUM bank).
"""

import math
import os
import sys

import numpy as np

for _p in ("/opt/trn_rl_repo", "/opt/pypackages"):
    if _p not in sys.path and os.path.isdir(_p):
        sys.path.append(_p)

# Problem dims (hardcoded per spec)
B, L, C, H = 4, 2048, 1024, 16
HD = C // H            # 64 head dim
NCORES = 8
EPS = 1e-5
P = 128                # partitions
NF = 512               # matmul moving free-dim tile
NCH = C // P           # 8 feature chunks
NLT = L // P           # 16 sequence tiles
LQ = L // 2            # 1024 query rows per core
NIC = LQ // NF         # 2 i-chunks
NPAIR = H // 2         # 8 head pairs
VW = HD + 1            # 65 = value cols + ones column (softmax denominator)
F4 = 4 * C             # 4096
NFC = F4 // P          # 32 fc chunks
SCL = 1.0 / math.sqrt(HD)
WS = 64.0              # fp8 weight pre-scale
SIGMA = 0.0            # Schraudolph tuning constant
# per-i-chunk jp subsets (of NLT//2) whose exp runs on DVE via the integer
# approximation; chunk 1 leans more on DVE because ACT also carries the
# concurrent MLP gelu of chunk 0.
DVE_JP_IC = ((1, 3, 6), (1, 3, 5, 6))
GELU_FUNC = "Gelu_apprx_tanh"

_CACHE = {}


def _build_bass():
    import concourse.bass as bass
    import concourse.mybir as mybir
    import concourse.tile as tile
    from concourse import bacc
    from concourse.bass import ts

    dt = mybir.dt
    f32, bf16 = dt.float32, dt.bfloat16
    AF = mybir.ActivationFunctionType
    OP = mybir.AluOpType

    nc = bacc.Bacc()

    hsT = nc.dram_tensor("hst", [C, L], bf16, kind="ExternalInput")
    xp8d = nc.dram_tensor("xp8", [NCH // 2, P, 2, L], dt.float8e4, kind="ExternalInput")
    hpRb = nc.dram_tensor("hprb", [C, LQ], f32, kind="ExternalInput")
    cvv = nc.dram_tensor("cvv", [C], f32, kind="ExternalInput")
    f8 = dt.float8e4
    f8l = dt.float8e5
    i8 = dt.int8
    DR = mybir.MatmulPerfMode.DoubleRow
    # fp8 DoubleRow layouts: [out_tile, partition(k), ktile_pair, 2, out_cols]
    wq8 = nc.dram_tensor("wq8", [NPAIR, P, NCH // 2, 2, P], f8, kind="ExternalInput")
    wk8 = nc.dram_tensor("wk8", [NPAIR, P, NCH // 2, 2, P], f8, kind="ExternalInput")
    # moving-operand layout for V: [partition(k), ktile_pair, 2, out_features]
    wvv8 = nc.dram_tensor("wvv8", [P, NCH // 2, 2, C], f8, kind="ExternalInput")
    wcp8 = nc.dram_tensor("wcp8", [NCH, P, NCH // 2, 2, P], f8, kind="ExternalInput")
    wfc8 = nc.dram_tensor("wfc8", [NFC, P, NCH // 2, 2, P], f8, kind="ExternalInput")
    wfc8l = nc.dram_tensor("wfc8l", [NFC, P, NCH // 2, 2, P], f8l, kind="ExternalInput")
    wproj8 = nc.dram_tensor("wproj8", [NCH, P, NFC // 2, 2, P], f8, kind="ExternalInput")
    wproj8l = nc.dram_tensor("wproj8l", [NCH, P, NFC // 2, 2, P], f8l,
                             kind="ExternalInput")
    bq = nc.dram_tensor("bq", [C], f32, kind="ExternalInput")
    bk = nc.dram_tensor("bk", [C], f32, kind="ExternalInput")
    bvv = nc.dram_tensor("bvv", [C], f32, kind="ExternalInput")
    bfc = nc.dram_tensor("bfc", [F4], f32, kind="ExternalInput")
    bprojb = nc.dram_tensor("bprojb", [C], bf16, kind="ExternalInput")
    outT = nc.dram_tensor("outt", [C, LQ], f32, kind="ExternalOutput")

    EXPA = SCL * 8.0 / math.log(2.0)
    EXPB = 56.0 + SIGMA

    with tile.TileContext(nc) as tc:
        with (
            tc.tile_pool(name="consts", bufs=1) as consts,
            tc.tile_pool(name="dram", bufs=1, space="DRAM") as dram,
        ):
            # --- constants ---
            ones_bf = consts.tile([P, 1], bf16)
            nc.vector.memset(ones_bf, 1.0)
            eps_sb = consts.tile([P, 1], f32)
            nc.vector.memset(eps_sb, EPS)
            eps64_sb = consts.tile([P, 1], f32)
            nc.vector.memset(eps64_sb, EPS * WS * WS)
            ones_row = consts.tile([1, NF], bf16)
            nc.vector.memset(ones_row, 1.0)
            bq_sb = consts.tile([P, NCH], f32)
            nc.sync.dma_start(out=bq_sb, in_=bq.rearrange("(o p) -> p o", p=P))
            bk_sb = consts.tile([P, NCH], f32)
            nc.sync.dma_start(out=bk_sb, in_=bk.rearrange("(o p) -> p o", p=P))
            bfc_sb = consts.tile([P, NFC], f32)
            nc.sync.dma_start(out=bfc_sb, in_=bfc.rearrange("(o p) -> p o", p=P))
            # bproj*64 laid out [1, C] for the PE bias-row trick
            bprow = consts.tile([1, C], bf16)
            nc.sync.dma_start(out=bprow, in_=bprojb[None, :])

            # ---------- plain LN: transposed stats + apply ----------
            def plain_ln(xpairs, Lx, statpool, bcpool, pspool, sqpool, tag,
                         out8=None, sq_engine="dve", ps_tag=None):
                """xpairs: NCH/2 SBUF bf16 pair tiles [P, 2, Lx]; normalized
                in place as x*rstd - mu*rstd. If out8 is given (NCH/2 fp8 pair
                tiles [P, 2, Lx]), the final subtract writes fp8 there
                directly (xpairs keep x*rstd).

                Stats are computed in transposed layout: matmuls with the
                x-slice as the stationary operand and a ones column as the
                moving operand produce [128, 1] per-position sums (nearly free
                on the PE: cost scales with output free size)."""
                nsl = Lx // P
                ps_sum = pspool.tile([P, NF], f32, tag=ps_tag or "pssum",
                                     name=f"pss_{tag}")
                ps_sq = pspool.tile([P, NF], f32, tag=ps_tag or "pssq",
                                    name=f"psq_{tag}")
                for cp in range(NCH // 2):
                    sq = sqpool.tile([P, 2, Lx], bf16, tag="sq", name=f"sq_{tag}_{cp}")
                    if sq_engine == "act":
                        for t_ in range(2):
                            nc.scalar.activation(out=sq[:, t_, :],
                                                 in_=xpairs[cp][:, t_, :],
                                                 func=AF.Square)
                    elif sq_engine == "pool":
                        nc.gpsimd.tensor_mul(sq, xpairs[cp], xpairs[cp])
                    else:
                        nc.vector.tensor_mul(sq, xpairs[cp], xpairs[cp])
                    for t in range(2):
                        c = 2 * cp + t
                        for s in range(nsl):
                            nc.tensor.matmul(
                                ps_sum[:, s:s + 1], xpairs[cp][:, t, ts(s, P)],
                                ones_bf,
                                start=(c == 0 and s == 0),
                                stop=(c == NCH - 1 and s == nsl - 1),
                                skip_group_check=True,
                            )
                            nc.tensor.matmul(
                                ps_sq[:, s:s + 1], sq[:, t, ts(s, P)], ones_bf,
                                start=(c == 0 and s == 0),
                                stop=(c == NCH - 1 and s == nsl - 1),
                                skip_group_check=True,
                            )
                # post: mu, rstd, mu*rstd on [P, nsl] tiles
                t = statpool.tile([P, 2, nsl], f32, tag="stat", name=f"t_{tag}")
                nc.scalar.activation(out=t[:, 0, :], in_=ps_sum[:, 0:nsl],
                                     func=AF.Copy, scale=1.0 / C)
                nc.vector.tensor_mul(t[:, 1, :], t[:, 0, :], t[:, 0, :])
                nc.vector.scalar_tensor_tensor(
                    out=t[:, 1, :], in0=ps_sq[:, 0:nsl], scalar=1.0 / C,
                    in1=t[:, 1, :], op0=OP.mult, op1=OP.subtract,
                )
                nc.scalar.activation(out=t[:, 1, :], in_=t[:, 1, :], func=AF.Sqrt,
                                     bias=eps_sb)
                nc.vector.reciprocal(t[:, 1, :], t[:, 1, :])      # rstd
                nc.vector.tensor_mul(t[:, 0, :], t[:, 0, :], t[:, 1, :])  # mu*rstd
                tb = statpool.tile([P, 2, nsl], bf16, tag="statb", name=f"tb_{tag}")
                nc.vector.tensor_copy(tb, t)
                # transpose-bounce through DRAM, then partition-broadcast
                ab_d = dram.tile([2, Lx], bf16, name=f"ab_d_{tag}")
                nc.sync.dma_start(out=ab_d.rearrange("a (s p) -> p a s", p=P),
                                  in_=tb)
                murs_bc = bcpool.tile([P, Lx], bf16, tag="abc", name=f"mursbc_{tag}")
                nc.sync.dma_start(out=murs_bc, in_=ab_d[0:1, :].broadcast_to([P, Lx]))
                rs_bc = bcpool.tile([P, Lx], bf16, tag="abc", name=f"rsbc_{tag}")
                nc.sync.dma_start(out=rs_bc, in_=ab_d[1:2, :].broadcast_to([P, Lx]))
                for cp in range(NCH // 2):
                    for t in range(2):
                        xs = xpairs[cp][:, t, :]
                        nc.vector.tensor_mul(xs, xs, rs_bc)
                        if out8 is None:
                            nc.vector.tensor_sub(xs, xs, murs_bc)
                        else:
                            with nc.allow_low_precision(reason="fp8 ln output"):
                                nc.vector.tensor_sub(out8[cp][:, t, :], xs,
                                                     murs_bc)
                return xpairs

            # long-lived pools, allocated in lifetime order (LIFO release)
            ktpool = tc.alloc_tile_pool(name="ktp", bufs=NPAIR)
            qtpool = tc.alloc_tile_pool(name="qtp", bufs=NPAIR)
            vnpool = tc.alloc_tile_pool(name="vnp", bufs=NLT // 2)
            vn_tiles = []

            with tc.tile_pool(name="lns8p", bufs=NCH // 2) as lns8p:
                lns8 = [lns8p.tile([P, 2, L], f8, tag="lns8", name=f"lns8_{cp}")
                        for cp in range(NCH // 2)]
                with tc.tile_pool(name="xp8p", bufs=NCH // 2) as xp8p:
                    # ---------- phase 1+2: hp stats (LN folded into V's
                    # output scales) + LN of hs ----------
                    wvvpool = tc.alloc_tile_pool(name="wvvp", bufs=1)
                    # thp outlives the phase-1 stat pools (feeds phase 3)
                    thppool = tc.alloc_tile_pool(name="thpp", bufs=1)
                    bcvpool = tc.alloc_tile_pool(name="bcvp", bufs=2)
                    with (
                        tc.tile_pool(name="lnps", bufs=NCH // 2) as lnps,
                        tc.tile_pool(name="sqp", bufs=2) as sqpool,
                        tc.tile_pool(name="statp", bufs=2) as statpool,
                        tc.tile_pool(name="bcp", bufs=2) as bcpool,
                        tc.tile_pool(name="psstat", bufs=2, space="PSUM") as pspool,
                    ):
                        xp_sb = []
                        for cp in range(NCH // 2):
                            xc = xp8p.tile([P, 2, L], f8, tag="xp8", name=f"xp_{cp}")
                            nc.sync.dma_start(out=xc, in_=xp8d[cp])
                            xp_sb.append(xc)
                        hs_pairs = []
                        for cp in range(NCH // 2):
                            xc = lnps.tile([P, 2, L], bf16, tag="lnh", name=f"hs_{cp}")
                            nc.sync.dma_start(
                                out=xc,
                                in_=hsT[ts(cp, 2 * P), :].rearrange(
                                    "(t p) l -> p t l", p=P))
                            hs_pairs.append(xc)
                        # prefetch the V weights while the inputs stream in
                        wvv_sb = wvvpool.tile([P, NCH // 2, 2, C], f8, tag="wvv",
                                              name="wvv_sb")
                        nc.sync.dma_start(out=wvv_sb, in_=wvv8[:, :, :, :])
                        bvv_bc = bcvpool.tile([P, C], f32, tag="bcv", name="bvv_bc")
                        cvv_bc = bcvpool.tile([P, C], f32, tag="bcv", name="cvv_bc")
                        nc.sync.dma_start(out=bvv_bc,
                                          in_=bvv[None, :].broadcast_to([P, C]))
                        nc.sync.dma_start(out=cvv_bc,
                                          in_=cvv[None, :].broadcast_to([P, C]))
                        # transposed stats for hp (no broadcast needed: the
                        # per-column scales land on V's output partitions)
                        nslh = L // P
                        pshs = pspool.tile([P, NF], f32, tag="pssum", name="pss_hp")
                        pshq = pspool.tile([P, NF], f32, tag="pssq", name="psq_hp")
                        for cp in range(NCH // 2):
                            sq = sqpool.tile([P, 2, L], bf16, tag="sq",
                                             name=f"sq_hp_{cp}")
                            for t_ in range(2):
                                nc.scalar.activation(out=sq[:, t_, :],
                                                     in_=xp_sb[cp][:, t_, :],
                                                     func=AF.Square)
                            for t in range(2):
                                c = 2 * cp + t
                                for sl in range(nslh):
                                    nc.tensor.matmul(
                                        pshs[:, sl:sl + 1],
                                        xp_sb[cp][:, t, ts(sl, P)], ones_bf,
                                        start=(c == 0 and sl == 0),
                                        stop=(c == NCH - 1 and sl == nslh - 1),
                                        skip_group_check=True,
                                    )
                                    nc.tensor.matmul(
                                        pshq[:, sl:sl + 1], sq[:, t, ts(sl, P)],
                                        ones_bf,
                                        start=(c == 0 and sl == 0),
                                        stop=(c == NCH - 1 and sl == nslh - 1),
                                        skip_group_check=True,
                                    )
                        thp = thppool.tile([P, 2, nslh], f32, tag="thp",
                                           name="t_hp")
                        nc.scalar.activation(out=thp[:, 0, :], in_=pshs[:, 0:nslh],
                                             func=AF.Copy, scale=1.0 / C)
                        nc.vector.tensor_mul(thp[:, 1, :], thp[:, 0, :],
                                             thp[:, 0, :])
                        nc.vector.scalar_tensor_tensor(
                            out=thp[:, 1, :], in0=pshq[:, 0:nslh], scalar=1.0 / C,
                            in1=thp[:, 1, :], op0=OP.mult, op1=OP.subtract,
                        )
                        # rstd/64: sqrt(64^2*(var+eps)) then reciprocal; the
                        # cvv sent by the host is pre-scaled by 64 to match.
                        nc.scalar.activation(out=thp[:, 1, :], in_=thp[:, 1, :],
                                             func=AF.Sqrt, scale=WS * WS,
                                             bias=eps64_sb)
                        nc.vector.reciprocal(thp[:, 1, :], thp[:, 1, :])
                        nc.vector.tensor_mul(thp[:, 0, :], thp[:, 0, :],
                                             thp[:, 1, :])
                        plain_ln(hs_pairs, L, statpool, bcpool, pspool,
                                 sqpool, "hs", out8=lns8, sq_engine="act")

                        # ---------- phase 3: V (fp8 DoubleRow on raw x, LN
                        # applied to the output rows) ----------
                        with (
                            tc.tile_pool(name="w2p", bufs=2) as w2pool,
                            tc.tile_pool(name="vps", bufs=2, space="PSUM") as vpspool,
                        ):
                            for lt in range(NLT):
                                vps = vpspool.tile([P, C], f32, tag="vps",
                                                   name=f"vps_{lt}")
                                for cp in range(NCH // 2):
                                    for dn in range(C // NF):
                                        nc.tensor.matmul(
                                            vps[:, ts(dn, NF)],
                                            xp_sb[cp][:, :, ts(lt, P)],
                                            wvv_sb[:, cp, :, ts(dn, NF)],
                                            start=(cp == 0),
                                            stop=(cp == NCH // 2 - 1),
                                            perf_mode=DR,
                                        )
                                if lt % 2 == 0:
                                    vnp = vnpool.tile([P, 2, H, VW], f8, tag="vn",
                                                      name=f"vn_{lt // 2}")
                                    nc.vector.memset(vnp[:, :, :, HD:VW], 1.0)
                                    vn_tiles.append(vnp)
                                # w2 = cvv64*murs64 - bvv; vn = vps*(rstd/64) - w2
                                w2 = w2pool.tile([P, C], f32, tag="w2",
                                                 name=f"w2_{lt}")
                                nc.vector.scalar_tensor_tensor(
                                    out=w2, in0=cvv_bc,
                                    scalar=thp[:, 0, lt:lt + 1],
                                    in1=bvv_bc, op0=OP.mult, op1=OP.subtract,
                                )
                                with nc.allow_low_precision(reason="fp8 v"):
                                    nc.vector.scalar_tensor_tensor(
                                        out=vn_tiles[lt // 2][:, lt % 2, :, 0:HD],
                                        in0=vps.rearrange("p (h d) -> p h d", d=HD),
                                        scalar=thp[:, 1, lt:lt + 1],
                                        in1=w2.rearrange("p (h d) -> p h d", d=HD),
                                        op0=OP.mult, op1=OP.subtract,
                                    )
                    bcvpool.release()
                    thppool.release()
                    wvvpool.release()
                # xp8p / lnps released here

                # ---------- phase 3.5: K/Q for ALL pairs (ACT evacuation) ----
                kt_tiles, qt_tiles = [], []
                with (
                    tc.tile_pool(name="wtkq", bufs=4) as wkqpool,
                    tc.tile_pool(name="kqps", bufs=2, space="PSUM") as kqpspool,
                ):
                    for pr in range(NPAIR):
                        kt = ktpool.tile([P, L], bf16, tag="kt", name=f"kt_{pr}")
                        wkt = wkqpool.tile([P, NCH // 2, 2, P], f8, tag="wtkq",
                                           name=f"wkt_{pr}")
                        nc.sync.dma_start(out=wkt, in_=wk8[pr])
                        for r in range(L // NF):
                            ps = kqpspool.tile([P, NF], f32, tag="kqps",
                                               name=f"kps_{pr}_{r}")
                            for cp in range(NCH // 2):
                                nc.tensor.matmul(
                                    ps, wkt[:, cp, :, :], lns8[cp][:, :, ts(r, NF)],
                                    start=(cp == 0), stop=(cp == NCH // 2 - 1),
                                    perf_mode=DR,
                                )
                            nc.scalar.activation(
                                out=kt[:, ts(r, NF)], in_=ps, func=AF.Identity,
                                scale=1.0 / WS, bias=bk_sb[:, pr:pr + 1])
                        qt = qtpool.tile([P, LQ], bf16, tag="qt", name=f"qt_{pr}")
                        wqt = wkqpool.tile([P, NCH // 2, 2, P], f8, tag="wtkq",
                                           name=f"wqt_{pr}")
                        nc.sync.dma_start(out=wqt, in_=wq8[pr])
                        for r in range(NIC):
                            ps = kqpspool.tile([P, NF], f32, tag="kqps",
                                               name=f"qps_{pr}_{r}")
                            for cp in range(NCH // 2):
                                nc.tensor.matmul(
                                    ps, wqt[:, cp, :, :], lns8[cp][:, :, ts(r, NF)],
                                    start=(cp == 0), stop=(cp == NCH // 2 - 1),
                                    perf_mode=DR,
                                )
                            nc.scalar.activation(
                                out=qt[:, ts(r, NF)], in_=ps, func=AF.Identity,
                                scale=1.0 / WS, bias=bq_sb[:, pr:pr + 1])
                        kt_tiles.append(kt)
                        qt_tiles.append(qt)
            # lns8 released here

            # tail pools allocated only now, in the address space freed by
            # the head phase working set
            h1pool = tc.alloc_tile_pool(name="h1p", bufs=NCH * NIC)
            ytpool = tc.alloc_tile_pool(name="ytp", bufs=NPAIR * NIC)
            wpool3 = tc.alloc_tile_pool(name="wt3", bufs=NCH)

            # prefetch the Wcp tiles so the first tail starts immediately
            wcp_sb = []
            for oc in range(NCH):
                wct = wpool3.tile([P, NCH // 2, 2, P], f8, tag="wt3",
                                  name=f"wcpt_{oc}")
                nc.sync.dma_start(out=wct, in_=wcp8[oc])
                wcp_sb.append(wct)
            # y tiles, one per (pair, i-chunk) so chunk tails never falsely
            # depend on the other chunk's attention writes
            yT = [[ytpool.tile([P, 2, NF], f8, tag="yth", name=f"yth_{pp}_{ic}")
                   for ic in range(NIC)] for pp in range(NPAIR // 2)]

            # ---------- attention (ic-outer) + per-chunk tails ----------
            with (
                tc.tile_pool(name="pup", bufs=6) as pupool,
                tc.tile_pool(name="recp", bufs=2) as recpool,
                tc.tile_pool(name="repsb", bufs=2) as reppool,
                tc.tile_pool(name="recd", bufs=4, space="DRAM") as recdpool,
                tc.tile_pool(name="stps", bufs=2, space="PSUM") as stpool,
                tc.tile_pool(name="yps", bufs=2, space="PSUM") as ypool,
                tc.tile_pool(name="tailps", bufs=2, space="PSUM") as tailps,
            ):
                for ic in range(NIC):
                    dve_set = DVE_JP_IC[ic]
                    for pr in range(NPAIR):
                        kt = kt_tiles[pr]
                        qt = qt_tiles[pr]
                        yth = yT[pr // 2][ic][:, pr % 2, :]
                        ypsA = ypool.tile([VW, NF], f32, tag="yps",
                                          name=f"ypsA_{pr}_{ic}")
                        ypsB = ypool.tile([VW, NF], f32, tag="yps",
                                          name=f"ypsB_{pr}_{ic}")
                        ypss = (ypsA, ypsB)
                        # software-pipelined jp loop: emit scores for round
                        # jp before the accumulates of round jp-1
                        prev = None
                        for jp in range(NLT // 2):
                            pus = []
                            for hh in range(2):
                                st = stpool.tile([P, 2, NF], f32, tag="st",
                                                 name=f"st_{pr}_{ic}_{jp}_{hh}")
                                for tp in range(2):
                                    j = 2 * jp + tp
                                    nc.tensor.matmul(
                                        st[:, tp, :],
                                        kt[ts(hh, HD), ts(j, P)],
                                        qt[ts(hh, HD), ts(ic, NF)],
                                        start=True, stop=True)
                                if jp in dve_set:
                                    pu_i = pupool.tile(
                                        [P, 2, NF], i8, tag="pu",
                                        name=f"pui_{pr}_{ic}_{jp}_{hh}")
                                    with nc.allow_low_precision(
                                            reason="approx exp bits"):
                                        nc.vector.tensor_scalar(
                                            out=pu_i, in0=st, scalar1=EXPA,
                                            scalar2=EXPB, op0=OP.mult,
                                            op1=OP.add)
                                    pus.append(pu_i.bitcast(f8))
                                else:
                                    pu = pupool.tile(
                                        [P, 2, NF], f8, tag="pu",
                                        name=f"pu_{pr}_{ic}_{jp}_{hh}")
                                    nc.scalar.activation(out=pu, in_=st,
                                                         func=AF.Exp,
                                                         scale=SCL)
                                    pus.append(pu)
                            if prev is not None:
                                pjp, ppus = prev
                                for hh in range(2):
                                    nc.tensor.matmul(
                                        ypss[hh],
                                        vn_tiles[pjp][:, :, 2 * pr + hh, :],
                                        ppus[hh],
                                        start=(pjp == 0),
                                        stop=(pjp == NLT // 2 - 1),
                                        perf_mode=DR)
                            prev = (jp, pus)
                        pjp, ppus = prev
                        for hh in range(2):
                            nc.tensor.matmul(
                                ypss[hh], vn_tiles[pjp][:, :, 2 * pr + hh, :],
                                ppus[hh],
                                start=(pjp == 0), stop=(pjp == NLT // 2 - 1),
                                perf_mode=DR)
                        for hh in range(2):
                            yps = ypss[hh]
                            rec = recpool.tile([1, NF], bf16, tag="rec",
                                               name=f"rec_{pr}_{ic}_{hh}")
                            with nc.allow_low_precision(
                                    reason="softmax denom reciprocal, bf16 ok"):
                                nc.vector.reciprocal(rec, yps[HD:VW, :])
                            # broadcast 1/den across the 64 head partitions
                            # via a DRAM bounce (no engine time, no PSUM bank)
                            recd = recdpool.tile([1, NF], bf16, tag="recd",
                                                 name=f"recd_{pr}_{ic}_{hh}")
                            nc.sync.dma_start(out=recd, in_=rec)
                            rep_sb = reppool.tile([HD, NF], bf16, tag="rep",
                                                  name=f"rep_{pr}_{ic}_{hh}")
                            nc.sync.dma_start(
                                out=rep_sb,
                                in_=recd[0:1, :].broadcast_to([HD, NF]))
                            # y_hi = numerator(PSUM) * rep(SBUF), fp8 out
                            with nc.allow_low_precision(reason="fp8 y"):
                                nc.vector.tensor_mul(
                                    yth[ts(hh, HD), :], yps[0:HD, :], rep_sb)

                    # ---------- tail for this i-chunk ----------
                    # Wcp (1-term) + residual
                    h1 = []
                    with tc.tile_pool(name=f"resp{ic}", bufs=3) as respool:
                        for oc in range(NCH):
                            cps = tailps.tile([P, NF], f32, tag="tail",
                                              name=f"cps_{ic}_{oc}")
                            for pp in range(NCH // 2):
                                nc.tensor.matmul(
                                    cps, wcp_sb[oc][:, pp, :, :],
                                    yT[pp][ic],
                                    start=(pp == 0), stop=(pp == NCH // 2 - 1),
                                    perf_mode=DR,
                                )
                            hp_r = respool.tile([P, NF], f32, tag="res",
                                                name=f"hpr_{ic}_{oc}")
                            nc.sync.dma_start(
                                out=hp_r, in_=hpRb[ts(oc, P), ts(ic, NF)])
                            h1c = h1pool.tile([P, NF], f32, tag="h1",
                                              name=f"h1_{ic}_{oc}")
                            # h1 = cps/64 + (hp + bcp)
                            nc.vector.scalar_tensor_tensor(
                                out=h1c, in0=cps, scalar=1.0 / WS, in1=hp_r,
                                op0=OP.mult, op1=OP.add,
                            )
                            h1.append(h1c)

                    # ln2 -> fp8 hi/lo pair tiles
                    with (
                        tc.tile_pool(name=f"ln2h{ic}", bufs=NCH // 2) as ln2hpool,
                        tc.tile_pool(name=f"ln2l{ic}", bufs=NCH // 2) as ln2lpool,
                    ):
                        with (
                            tc.tile_pool(name=f"ln2b{ic}", bufs=NCH // 2) as ln2bpool,
                            tc.tile_pool(name=f"sqp2{ic}", bufs=2) as sqpool2,
                            tc.tile_pool(name=f"statp2{ic}", bufs=2) as statpool2,
                            tc.tile_pool(name=f"bcp2{ic}", bufs=2) as bcpool2,
                        ):
                            h1_bf = []
                            for cp in range(NCH // 2):
                                hb = ln2bpool.tile([P, 2, NF], bf16, tag="ln2b",
                                                   name=f"h1b_{ic}_{cp}")
                                for t in range(2):
                                    nc.gpsimd.tensor_copy(hb[:, t, :],
                                                          h1[2 * cp + t])
                                h1_bf.append(hb)
                            ln2 = plain_ln(h1_bf, NF, statpool2, bcpool2,
                                           tailps, sqpool2, f"l2_{ic}",
                                           sq_engine="pool", ps_tag="tail")
                            ln2h, ln2l = [], []
                            for cp in range(NCH // 2):
                                lh = ln2hpool.tile([P, 2, NF], f8, tag="ln2h",
                                                   name=f"ln2h_{ic}_{cp}")
                                ll = ln2lpool.tile([P, 2, NF], f8l, tag="ln2l",
                                                   name=f"ln2l_{ic}_{cp}")
                                nc.scalar.activation(out=lh, in_=ln2[cp],
                                                     func=AF.Copy)
                                with nc.allow_low_precision(
                                        reason="fp8 lo residual"):
                                    nc.vector.tensor_sub(ll, ln2[cp], lh)
                                ln2h.append(lh)
                                ln2l.append(ll)

                        # MLP (fp8 DoubleRow, 3-term hi/lo)
                        with (
                            tc.tile_pool(name=f"fchp{ic}", bufs=NFC // 2) as fchpool,
                            tc.tile_pool(name=f"fclp{ic}", bufs=NFC // 2) as fclpool,
                        ):
                            fc8h, fc8l = [], []
                            wpool5 = tc.alloc_tile_pool(name=f"wt5{ic}", bufs=2)
                            wproj_pre = {}
                            with (
                                tc.tile_pool(name=f"wt4{ic}", bufs=4) as wpool4,
                                tc.tile_pool(name=f"fcb{ic}", bufs=3) as fcbpool,
                            ):
                                wpt0 = wpool5.tile([P, NFC // 2, 2, P], f8,
                                                   tag="wt5", name=f"wprt_{ic}_0")
                                nc.sync.dma_start(out=wpt0, in_=wproj8[0])
                                wptl0 = wpool5.tile([P, NFC // 2, 2, P], f8l,
                                                    tag="wt5", name=f"wprtl_{ic}_0")
                                nc.sync.dma_start(out=wptl0, in_=wproj8l[0])
                                wproj_pre[0] = (wpt0, wptl0)
                                for fo in range(NFC):
                                    wft = wpool4.tile([P, NCH // 2, 2, P], f8,
                                                      tag="wt4", name=f"wfct_{ic}_{fo}")
                                    nc.sync.dma_start(out=wft, in_=wfc8[fo])
                                    wftl = wpool4.tile([P, NCH // 2, 2, P], f8l,
                                                       tag="wt4",
                                                       name=f"wfctl_{ic}_{fo}")
                                    nc.sync.dma_start(out=wftl, in_=wfc8l[fo])
                                    fps = tailps.tile([P, NF], f32, tag="tail",
                                                      name=f"fps_{ic}_{fo}")
                                    for term, (wt_, act_) in enumerate(
                                            ((wft, ln2h), (wftl, ln2h),
                                             (wft, ln2l))):
                                        for cp in range(NCH // 2):
                                            nc.tensor.matmul(
                                                fps, wt_[:, cp, :, :],
                                                act_[cp],
                                                start=(term == 0 and cp == 0),
                                                stop=(term == 2 and
                                                      cp == NCH // 2 - 1),
                                                perf_mode=DR,
                                            )
                                    if fo % 2 == 0:
                                        fc8h.append(fchpool.tile(
                                            [P, 2, NF], f8, tag="fch",
                                            name=f"fch_{ic}_{fo // 2}"))
                                        fc8l.append(fclpool.tile(
                                            [P, 2, NF], f8l, tag="fcl",
                                            name=f"fcl_{ic}_{fo // 2}"))
                                    fcb = fcbpool.tile([P, NF], bf16, tag="fcb",
                                                       name=f"fcb_{ic}_{fo}")
                                    # gelu((fps/64) + bfc) -> bf16 master
                                    nc.scalar.activation(
                                        out=fcb, in_=fps,
                                        func=getattr(AF, GELU_FUNC),
                                        scale=1.0 / WS,
                                        bias=bfc_sb[:, fo:fo + 1])
                                    with nc.allow_low_precision(reason="fp8 hi/lo"):
                                        nc.gpsimd.tensor_copy(
                                            fc8h[fo // 2][:, fo % 2, :], fcb)
                                        nc.vector.tensor_sub(
                                            fc8l[fo // 2][:, fo % 2, :], fcb,
                                            fc8h[fo // 2][:, fo % 2, :])

                            with tc.tile_pool(name=f"outp{ic}", bufs=3) as opool:
                                for oc in range(NCH):
                                    if oc in wproj_pre:
                                        wpt, wptl = wproj_pre[oc]
                                    else:
                                        wpt = wpool5.tile(
                                            [P, NFC // 2, 2, P], f8, tag="wt5",
                                            name=f"wprt_{ic}_{oc}")
                                        nc.sync.dma_start(out=wpt, in_=wproj8[oc])
                                        wptl = wpool5.tile(
                                            [P, NFC // 2, 2, P], f8l, tag="wt5",
                                            name=f"wprtl_{ic}_{oc}")
                                        nc.sync.dma_start(out=wptl, in_=wproj8l[oc])
                                    pps = tailps.tile([P, NF], f32, tag="tail",
                                                      name=f"pps_{ic}_{oc}")
                                    # bias row: bproj*64 enters the PSUM group
                                    # via a rank-1 matmul (frees a DVE op)
                                    nc.tensor.matmul(
                                        pps, bprow[:, ts(oc, P)], ones_row,
                                        start=True, stop=False,
                                        skip_group_check=True)
                                    for term, (wt_, act_) in enumerate(
                                            ((wpt, fc8h), (wptl, fc8h),
                                             (wpt, fc8l))):
                                        for fp in range(NFC // 2):
                                            nc.tensor.matmul(
                                                pps, wt_[:, fp, :, :],
                                                act_[fp],
                                                start=False,
                                                stop=(term == 2 and
                                                      fp == NFC // 2 - 1),
                                                perf_mode=DR,
                                                skip_group_check=True,
                                            )
                                    osb = opool.tile([P, NF], f32, tag="osb",
                                                     name=f"osb_{ic}_{oc}")
                                    # out = pps/64 + h1   (pps includes bproj*64)
                                    nc.vector.scalar_tensor_tensor(
                                        out=osb, in0=pps, scalar=1.0 / WS,
                                        in1=h1[oc], op0=OP.mult, op1=OP.add,
                                    )
                                    nc.sync.dma_start(
                                        out=outT[ts(oc, P), ts(ic, NF)], in_=osb)
                            wpool5.release()

            wpool3.release()
            ytpool.release()
            h1pool.release()
            vnpool.release()
            qtpool.release()
            ktpool.release()

    nc.finalize()
    return nc


def _host_prep(inputs):
    """Fold weights on host; returns dict of shared (per-core-identical) arrays."""
    import ml_dtypes

    f64 = np.float64
    g = {k: np.asarray(v) for k, v in inputs.items()}
    Wv = g["Wv"].astype(f64)
    Wvt = g["Wvt"].astype(f64)
    Wvv = Wv @ Wvt
    bvv = g["bv"].astype(f64) @ Wvt + g["bvt"].astype(f64)

    def fold(w_ln, b_ln, W, bias):
        W = W.astype(f64)
        Wf = w_ln.astype(f64)[:, None] * W
        bf = b_ln.astype(f64) @ W + bias.astype(f64)
        return Wf, bf

    Wq_, bq_ = fold(g["ln1s_w"], g["ln1s_b"], g["Wq"], g["bq"])
    Wk_, bk_ = fold(g["ln1s_w"], g["ln1s_b"], g["Wk"], g["bk"])
    Wvv_, bvv_ = fold(g["ln1p_w"], g["ln1p_b"], Wvv, bvv)
    Wfc_, bfc_ = fold(g["ln2_w"], g["ln2_b"], g["Wfc"], g["bfc"])

    f8 = ml_dtypes.float8_e4m3
    f8e5 = ml_dtypes.float8_e5m2

    def dr_layout(W, d=None):
        """[K, N] -> [N/128, 128(k), K/256, 2, 128(n)] fp8 DoubleRow tiles."""
        K, N = W.shape
        Wt = W.reshape(K // 256, 2, 128, N // 128, 128)  # cp, t, k, ot, n
        return np.ascontiguousarray(Wt.transpose(3, 2, 0, 1, 4).astype(d or f8))

    def mv_layout(W):
        """[K, N] -> [128(k), K/256, 2, N] fp8 DoubleRow moving layout."""
        K, N = W.shape
        Wt = W.reshape(K // 256, 2, 128, N)  # cp, t, k, n
        return np.ascontiguousarray(Wt.transpose(2, 0, 1, 3).astype(f8))

    def lo(W):
        return W - W.astype(f8).astype(f64)

    WS_ = f64(WS)
    Wcp64 = g["Wcp"].astype(f64) * WS_
    Wproj64 = g["Wproj"].astype(f64) * WS_
    Wfc64 = Wfc_ * WS_
    Wvv64 = Wvv_ * WS_
    # cvv: column sums of the quantized, scaled V weights as the PE sees them
    cvv64 = Wvv64.astype(f8).astype(f64).sum(axis=0)
    return {
        "cvv": cvv64.astype(np.float32),
        "wq8": dr_layout(Wq_ * WS_),
        "wk8": dr_layout(Wk_ * WS_),
        "wvv8": mv_layout(Wvv64),
        "wcp8": dr_layout(Wcp64),
        "wfc8": dr_layout(Wfc64),
        "wfc8l": dr_layout(lo(Wfc64), f8e5),
        "wproj8": dr_layout(Wproj64),
        "wproj8l": dr_layout(lo(Wproj64), f8e5),
        "bq": bq_.astype(np.float32),
        "bk": bk_.astype(np.float32),
        "bvv": bvv_.astype(np.float32),
        "bfc": bfc_.astype(np.float32),
        "bprojb": (g["bproj"].astype(f64) * WS_).astype(ml_dtypes.bfloat16),
    }


def kernel(**inputs):
    from concourse.bass_utils import run_bass_kernel_spmd

    attn_dim = int(np.asarray(inputs["attn_dim"]))
    assert attn_dim in (-2, 1), f"unsupported attn_dim {attn_dim}"

    h_private = np.asarray(inputs["h_private"], dtype=np.float32)
    h_shared = np.asarray(inputs["h_shared"], dtype=np.float32)

    shared_ins = _host_prep(inputs)
    bcp = np.asarray(inputs["bcp"], dtype=np.float32)

    import ml_dtypes

    bf16 = ml_dtypes.bfloat16
    f8 = ml_dtypes.float8_e4m3
    in_maps = []
    for core in range(NCORES):
        b, s = divmod(core, 2)
        roll = s * LQ
        hs = np.concatenate([h_shared[b][roll:], h_shared[b][:roll]], axis=0)
        hp = np.concatenate([h_private[b][roll:], h_private[b][:roll]], axis=0)
        m = dict(shared_ins)
        m["hst"] = np.ascontiguousarray(hs.T.astype(bf16))
        m["xp8"] = np.ascontiguousarray(
            hp.T.reshape(NCH // 2, 2, P, L).transpose(0, 2, 1, 3).astype(f8))
        m["hprb"] = np.ascontiguousarray(
            h_private[b][s * LQ:(s + 1) * LQ].T + bcp[:, None])
        in_maps.append(m)

    if "nc" not in _CACHE:
        _CACHE["nc"] = _build_bass()
    nc = _CACHE["nc"]

    res = run_bass_kernel_spmd(nc, in_maps, list(range(NCORES)))
    _CACHE["last_res"] = res

    out = np.empty((B, L, C), np.float32)
    for core in range(NCORES):
        b, s = divmod(core, 2)
        out[b, s * LQ:(s + 1) * LQ, :] = res.results[core]["outt"].T
    return out, h_shared
